# revision 1
# baseline (speedup 1.0000x reference)
"""Trainium2 Bass kernel for nn_AbsGlobalHeadProbEncoder (MFVI message passing).

kernel(**inputs) takes the FULL inputs
    x       [4, 1024, 128] f32
    mask    [4, 1024]      i32   (all ones per the problem spec)
    ternary [128, 128, 8]  f32
    global_ [64, 128, 8]   f32
and returns the FULL output [4, 1024, 128] f32.

Sharding: 8 NeuronCores, one batch element per core pair (cores 2n and 2n+1
redundantly compute batch n with all 8 heads). A cross-core AllReduce variant
was measured at ~2.2 ms per 512 KB pairwise all-reduce on this stack - far
more than the ~0.15 ms of per-iteration compute it would save - so full
replication wins.

Per core and MFVI iteration: scores F_H = [H, L, L+G] are built chunk-wise
with fp16 matmuls into fp32 PSUM, softmax'd with a fused exp+rowsum on the
scalar engine, normalized by a per-partition DVE scale, transposed on the PE
(fp16, packed PSUM banks) for the msg_i contraction, and all three messages
accumulate into one shared [D, L] fp32 PSUM region.
"""
import sys
import contextlib

if '/opt/trn_rl_repo' not in sys.path:
    sys.path.insert(0, '/opt/trn_rl_repo')

import numpy as np
import concourse.bacc as bacc
import concourse.mybir as mybir
import concourse.tile as tile
from concourse.masks import make_identity

F32 = mybir.dt.float32
F16 = mybir.dt.float16
AF = mybir.ActivationFunctionType

B = 4
L = 1024
D = 128
H = 8
G = 64
IC = L // 128
N_CORES = 8

_CACHE = {}


def build_kernel(n_iter=4, num_devices=8, groups=None, hpc=4, sb_bufs=2, use_cc=True):
    if groups is None:
        groups = [[2 * i, 2 * i + 1] for i in range(num_devices // 2)]
    nc = bacc.Bacc("TRN2", target_bir_lowering=False, debug=False,
                   num_devices=num_devices)

    xT = nc.declare_dram_parameter("xT", [D, L], F32, isOutput=False)
    tern_a = nc.declare_dram_parameter("tern_a", [D, hpc * D], F16, isOutput=False)
    tern_b = nc.declare_dram_parameter("tern_b", [D, hpc * D], F16, isOutput=False)
    glT = nc.declare_dram_parameter("glT", [D, hpc * G], F16, isOutput=False)
    gl2 = nc.declare_dram_parameter("gl2", [2 * G, (hpc // 2) * D], F16, isOutput=False)
    y = nc.declare_dram_parameter("y", [L, D], F32, isOutput=True)

    with tile.TileContext(nc) as tc:
        with contextlib.ExitStack() as ctx:
            singles = ctx.enter_context(tc.tile_pool(name="singles", bufs=1))
            sb = ctx.enter_context(tc.tile_pool(name="sb", bufs=sb_bufs))
            hp = ctx.enter_context(tc.tile_pool(name="hp", bufs=2))
            # PSUM budget (8 banks): msgp 2 + fh 2x2 + aux 2x1 = 8
            aux = ctx.enter_context(tc.tile_pool(name="aux", bufs=2, space="PSUM"))
            fh = ctx.enter_context(tc.tile_pool(name="fh", bufs=2, space="PSUM"))
            msgp = ctx.enter_context(tc.tile_pool(name="msgp", bufs=1, space="PSUM"))
            dram = ctx.enter_context(tc.tile_pool(name="dram", bufs=2, space="DRAM"))

            # ---- persistent SBUF state ----
            unaryT = singles.tile([D, L], F32)
            nc.sync.dma_start(unaryT[:], xT[:])
            ta_sb = singles.tile([D, hpc * D], F16)
            nc.sync.dma_start(ta_sb[:], tern_a[:])
            tb_sb = singles.tile([D, hpc * D], F16)
            nc.sync.dma_start(tb_sb[:], tern_b[:])
            glT_sb = singles.tile([D, hpc * G], F16)
            nc.sync.dma_start(glT_sb[:], glT[:])
            gl2_sb = singles.tile([2 * G, (hpc // 2) * D], F16)
            nc.sync.dma_start(gl2_sb[:], gl2[:])
            id16 = singles.tile([128, 128], F16)
            make_identity(nc, id16[:])
            id32 = singles.tile([128, 128], F32)
            make_identity(nc, id32[:])

            qzT = singles.tile([D, L], F16)
            fzT = singles.tile([D, L], F32)

            def z_tail(fzT_src, last=False):
                """qzT <- softmax_D(fzT_src^T)^T ; if last: y <- fzT_src^T."""
                if last:
                    out_sb = sb.tile([128, L], F32, tag="zout")
                    for ic in range(IC):
                        fz_ps = fh.tile([128, 128], F32, tag="fh_ps")
                        nc.tensor.transpose(fz_ps[:], fzT_src[:, ic * 128:(ic + 1) * 128], id32[:])
                        nc.vector.tensor_copy(out_sb[:, ic * 128:(ic + 1) * 128], fz_ps[:])
                        nc.sync.dma_start(y[ic * 128:(ic + 1) * 128, :],
                                          out_sb[:, ic * 128:(ic + 1) * 128])
                    return
                ez = sb.tile([128, L], F32, tag="ez")
                sums = sb.tile([128, IC], F32, tag="zsums")
                for ic in range(IC):
                    cs = slice(ic * 128, (ic + 1) * 128)
                    fz_ps = fh.tile([128, 128], F32, tag="fh_ps")
                    nc.tensor.transpose(fz_ps[:], fzT_src[:, cs], id32[:])
                    nc.scalar.activation(ez[:, cs], fz_ps[:], AF.Exp,
                                         accum_out=sums[:, ic:ic + 1])
                rz = sb.tile([128, IC], F32, tag="zrz")
                nc.vector.reciprocal(rz[:], sums[:])
                qz_sc = sb.tile([128, L], F16, tag="qzsc")
                for ic in range(IC):
                    cs = slice(ic * 128, (ic + 1) * 128)
                    nc.vector.tensor_scalar_mul(qz_sc[:, cs], ez[:, cs], rz[:, ic:ic + 1])
                qzT_ps = fh.tile([128, L], F16, tag="fh_ps")
                for ic in range(IC):
                    cs = slice(ic * 128, (ic + 1) * 128)
                    nc.tensor.transpose(qzT_ps[:, cs], qz_sc[:, cs], id16[:])
                nc.vector.tensor_copy(qzT[:], qzT_ps[:])

            z_tail(unaryT)

            for it in range(n_iter):
                # ---------- phase A + B, software-pipelined across heads ----------
                # Emission order drives each engine's program order: head h+1's
                # score build (PE) is emitted before head h's message matmuls so
                # the PE streams F(h+1) while ACT/DVE finish exp/normalize(h).
                s_sb = sb.tile([128, IC * hpc * 128], F16, tag="s_sb")
                r_sb = sb.tile([128, IC * hpc * 128], F16, tag="r_sb")
                eg_sb = sb.tile([128, IC * hpc * G], F16, tag="eg_sb")
                eg_sums = sb.tile([128, IC * hpc], F32, tag="eg_sums")
                msg_ps = msgp.tile([128, L], F32, tag="msg_ps")
                sts = [None] * hpc
                ebs = [None] * hpc
                sms = [None] * hpc

                def emit_front(h):
                    hs = slice(h * 128, (h + 1) * 128)
                    st_ps = fh.tile([128, L], F32, tag="fh_ps")
                    for half in range(2):
                        nc.tensor.matmul(st_ps[:, half * 512:(half + 1) * 512],
                                         ta_sb[:, hs], qzT[:, half * 512:(half + 1) * 512])
                    st_sb = hp.tile([128, L], F16, tag="st_sb")
                    nc.scalar.copy(st_sb[:], st_ps[:])
                    e_big = hp.tile([128, IC * L], F16, tag="e_big")
                    sums = hp.tile([128, IC], F32, tag="hsums")
                    for ic in range(IC):
                        fh_ps = fh.tile([128, L], F32, tag="fh_ps")
                        for half in range(2):
                            nc.tensor.matmul(fh_ps[:, half * 512:(half + 1) * 512],
                                             st_sb[:, ic * 128:(ic + 1) * 128],
                                             qzT[:, half * 512:(half + 1) * 512])
                        nc.scalar.activation(e_big[:, ic * L:(ic + 1) * L], fh_ps[:],
                                             AF.Exp, accum_out=sums[:, ic:ic + 1])
                    sts[h], ebs[h], sms[h] = st_sb, e_big, sums

                def emit_phase_a():
                    # s_sb copies on DVE so ACT keeps streaming head-0 exps
                    for c in range(IC):
                        cs = slice(c * 128, (c + 1) * 128)
                        os_ = slice(c * hpc * 128, (c + 1) * hpc * 128)
                        s_ps = fh.tile([128, hpc * 128], F32, tag="fh_ps")
                        for half in range(max(1, hpc * 128 // 512)):
                            nh = slice(half * 512, min((half + 1) * 512, hpc * 128))
                            nc.tensor.matmul(s_ps[:, nh], qzT[:, cs], ta_sb[:, nh])
                        nc.vector.tensor_copy(s_sb[:, os_], s_ps[:])
                    ics_per_bank = max(1, 512 // (hpc * G))
                    for ic2 in range(IC // ics_per_bank):
                        hg_ps = fh.tile([128, ics_per_bank * hpc * G], F32, tag="fh_ps")
                        for k in range(ics_per_bank):
                            ic = ics_per_bank * ic2 + k
                            nc.tensor.matmul(hg_ps[:, k * hpc * G:(k + 1) * hpc * G],
                                             qzT[:, ic * 128:(ic + 1) * 128], glT_sb[:])
                        nc.scalar.activation(
                            eg_sb[:, ic2 * ics_per_bank * hpc * G:(ic2 + 1) * ics_per_bank * hpc * G],
                            hg_ps[:], AF.Exp)
                    nc.vector.reduce_sum(eg_sums[:],
                                         eg_sb.rearrange("p (s g) -> p s g", g=G),
                                         axis=mybir.AxisListType.X)
                    for c in range(IC):
                        cs = slice(c * 128, (c + 1) * 128)
                        os_ = slice(c * hpc * 128, (c + 1) * hpc * 128)
                        r_ps = fh.tile([128, hpc * 128], F32, tag="fh_ps")
                        for half in range(max(1, hpc * 128 // 512)):
                            nh = slice(half * 512, min((half + 1) * 512, hpc * 128))
                            nc.tensor.matmul(r_ps[:, nh], qzT[:, cs], tb_sb[:, nh])
                        nc.scalar.copy(r_sb[:, os_], r_ps[:])

                def emit_back(h):
                    hs = slice(h * 128, (h + 1) * 128)
                    st_sb, e_big, sums = sts[h], ebs[h], sms[h]
                    et_big = hp.tile([128, IC * L], F16, tag="et_big")
                    tot = hp.tile([128, IC], F32, tag="htot")
                    rr = hp.tile([128, IC], F32, tag="hr")
                    eg_h_sums = eg_sums.rearrange("p (s h) -> p s h", h=hpc)[:, :, h]
                    nc.vector.tensor_add(tot[:], sums[:], eg_h_sums)
                    nc.vector.reciprocal(rr[:], tot[:])
                    for ic in range(IC):
                        es = slice(ic * L, (ic + 1) * L)
                        nc.vector.tensor_scalar_mul(e_big[:, es], e_big[:, es],
                                                    rr[:, ic:ic + 1])
                        for half in range(2):
                            nc.tensor.matmul(
                                msg_ps[:, half * 512:(half + 1) * 512],
                                s_sb[:, (ic * hpc + h) * 128:(ic * hpc + h + 1) * 128],
                                e_big[:, ic * L + half * 512: ic * L + (half + 1) * 512],
                                start=(h == 0 and ic == 0), stop=False)
                        t_ps = aux.tile([128, L], F16, tag="aux_ps")
                        for jc in range(IC):
                            nc.tensor.transpose(t_ps[:, jc * 128:(jc + 1) * 128],
                                                e_big[:, ic * L + jc * 128: ic * L + (jc + 1) * 128],
                                                id16[:])
                        cp_eng = nc.vector.tensor_copy
                        cp_eng(
                            et_big.rearrange("p (jc i) -> p jc i", jc=IC)[:, :, ic * 128:(ic + 1) * 128],
                            t_ps.rearrange("p (jc i) -> p jc i", jc=IC))
                    for jc in range(IC):
                        for half in range(2):
                            nc.tensor.matmul(
                                msg_ps[:, half * 512:(half + 1) * 512],
                                r_sb[:, (jc * hpc + h) * 128:(jc * hpc + h + 1) * 128],
                                et_big[:, jc * L + half * 512: jc * L + (half + 1) * 512],
                                start=False, stop=False)
                    for ic in range(IC):
                        col = (ic * hpc + h) * G
                        nc.vector.tensor_scalar_mul(eg_sb[:, col:col + G],
                                                    eg_sb[:, col:col + G], rr[:, ic:ic + 1])

                emit_front(0)
                emit_phase_a()
                for h in range(hpc):
                    if h + 1 < hpc:
                        emit_front(h + 1)
                    emit_back(h)

                # eg tail: head-pair-stacked transposes + 2G-deep msg_g matmuls
                for k in range(hpc // 2):
                    egT_ps = aux.tile([128, IC * 128], F16, tag="aux_ps")
                    for ic in range(IC):
                        col = (ic * hpc + 2 * k) * G
                        nc.tensor.transpose(egT_ps[:, ic * 128:(ic + 1) * 128],
                                            eg_sb[:, col:col + 2 * G], id16[:])
                    egT_sb = hp.tile([128, IC * 128], F16, tag="egT_sb")
                    nc.scalar.copy(egT_sb[:], egT_ps[:])
                    for half in range(2):
                        nc.tensor.matmul(msg_ps[:, half * 512:(half + 1) * 512],
                                         gl2_sb[:, k * 128:(k + 1) * 128],
                                         egT_sb[:, half * 512:(half + 1) * 512],
                                         start=False, stop=(k == hpc // 2 - 1))

                # ---------- phase C: all-reduce + Z update ----------
                if use_cc:
                    msg_sb = sb.tile([128, L], F32, tag="msg_sb")
                    nc.vector.tensor_copy(msg_sb[:], msg_ps[:])
                    bi = dram.tile([128, L], F32, tag="cc_in")
                    bo = dram.tile([128, L], F32, tag="cc_out")
                    nc.sync.dma_start(bi[:], msg_sb[:])
                    nc.gpsimd.collective_compute(
                        "AllReduce", mybir.AluOpType.add,
                        replica_groups=groups,
                        ins=[bi.opt()], outs=[bo.opt()])
                    msg_red = sb.tile([128, L], F32, tag="msg_red")
                    nc.sync.dma_start(msg_red[:], bo[:])
                    nc.vector.tensor_add(fzT[:], msg_red[:], unaryT[:])
                else:
                    nc.vector.tensor_add(fzT[:], msg_ps[:], unaryT[:])
                z_tail(fzT, last=(it == n_iter - 1))

    nc.compile()
    return nc

class _Runner:
    """Keeps the jitted SPMD executable alive across kernel() calls."""

    def __init__(self, nc):
        import jax
        from jax.sharding import Mesh, PartitionSpec
        from jax.experimental.shard_map import shard_map
        from concourse.bass2jax import (_bass_exec_p, install_neuronx_cc_hook,
                                        partition_id_tensor)
        install_neuronx_cc_hook()
        self.jax = jax
        in_names, out_names, out_avals, zero_outs = [], [], [], []
        partition_name = nc.partition_id_tensor.name if nc.partition_id_tensor else None
        for alloc in nc.m.functions[0].allocations:
            if not isinstance(alloc, mybir.MemoryLocationSet):
                continue
            name = alloc.memorylocations[0].name
            if alloc.kind == "ExternalInput":
                if name != partition_name:
                    in_names.append(name)
            elif alloc.kind == "ExternalOutput":
                out_names.append(name)
                shape = tuple(alloc.tensor_shape)
                dtype = mybir.dt.np(alloc.dtype)
                out_avals.append(jax.core.ShapedArray(shape, dtype))
                zero_outs.append(np.zeros(shape, dtype))
        self.in_names, self.out_names = in_names, out_names
        self.out_avals, self.zero_outs = out_avals, zero_outs
        all_in_names = list(in_names) + list(out_names)
        if partition_name is not None:
            all_in_names.append(partition_name)

        def _body(*args):
            operands = list(args)
            if partition_name is not None:
                operands.append(partition_id_tensor())
            outs = _bass_exec_p.bind(
                *operands,
                out_avals=tuple(out_avals),
                in_names=tuple(all_in_names),
                out_names=tuple(out_names),
                lowering_input_output_aliases=(),
                sim_require_finite=True,
                sim_require_nnan=True,
                nc=nc,
            )
            return tuple(outs)

        devices = jax.devices()[:N_CORES]
        mesh = Mesh(np.asarray(devices), ("core",))
        n_params = len(in_names)
        in_specs = (PartitionSpec("core"),) * (n_params + len(out_names))
        out_specs = (PartitionSpec("core"),) * len(out_names)
        self.fn = jax.jit(shard_map(_body, mesh=mesh, in_specs=in_specs,
                                    out_specs=out_specs, check_rep=False),
                          keep_unused=True)

    def __call__(self, in_maps):
        jax = self.jax
        concat_in = [
            np.concatenate([np.asarray(in_maps[c][name]) for c in range(N_CORES)], axis=0)
            for name in self.in_names
        ]
        concat_zeros = [np.zeros((N_CORES * z.shape[0], *z.shape[1:]), z.dtype)
                        for z in self.zero_outs]
        outs = self.fn(*concat_in, *concat_zeros)
        jax.block_until_ready(outs)
        return [
            {name: np.asarray(outs[i]).reshape(N_CORES, *self.out_avals[i].shape)[c]
             for i, name in enumerate(self.out_names)}
            for c in range(N_CORES)
        ]


def make_core_inputs(x, ternary, global_, core, hpc=8):
    n = core // 2
    if hpc == 8:
        heads = list(range(8))
    else:
        hg = core % 2
        heads = list(range(hg * hpc, (hg + 1) * hpc))
    t = ternary[:, :, heads]
    g = global_[:, :, heads]
    return {
        "xT": np.ascontiguousarray(x[n].T.astype(np.float32)),
        "tern_a": np.ascontiguousarray(t.transpose(0, 2, 1).reshape(D, hpc * D).astype(np.float16)),
        "tern_b": np.ascontiguousarray(t.transpose(1, 2, 0).reshape(D, hpc * D).astype(np.float16)),
        "glT": np.ascontiguousarray(g.transpose(1, 2, 0).reshape(D, hpc * G).astype(np.float16)),
        "gl2": np.ascontiguousarray(
            g.transpose(2, 0, 1).reshape(hpc // 2, 2, G, D)
             .transpose(1, 2, 0, 3).reshape(2 * G, (hpc // 2) * D).astype(np.float16)),
    }


def get_runner(n_iter=4):
    key = ("runner", n_iter)
    if key not in _CACHE:
        nc = build_kernel(n_iter=n_iter, num_devices=N_CORES, hpc=8, use_cc=False)
        _CACHE[key] = _Runner(nc)
    return _CACHE[key]


def kernel(x, mask, ternary, global_):
    x = np.asarray(x, dtype=np.float32)
    mask = np.asarray(mask)
    ternary = np.asarray(ternary, dtype=np.float32)
    global_ = np.asarray(global_, dtype=np.float32)

    run = get_runner(4)
    in_maps = [make_core_inputs(x, ternary, global_, c) for c in range(N_CORES)]
    res = run(in_maps)
    out = np.stack([res[2 * n]["y"] for n in range(B)])
    out = np.where((mask != 0)[..., None], out, np.float32(0.0)).astype(np.float32)
    return out



# revision 2
# speedup vs baseline: 1.2100x; 1.2100x over previous
"""Trainium2 Bass kernel v2 for nn_AbsGlobalHeadProbEncoder (MFVI message passing).

Sequence-parallel over the query (i) axis: 8 cores = 4 batch elements x 2
token-halves.  Each core computes head scores / messages only for its own 512
query rows (all 8 heads), so per-core PE work halves vs the replicated
baseline.  Cross-core per iteration:
  - ReduceScatter (fp16) of the partial msg_j [D, L] -> each core gets the
    fully-summed messages for its own token half.
  - AllGather (fp16) of the updated Qz^T half -> full qzT for the next
    iteration's scores.
Inputs are host-permuted so each core's own tokens are local columns 0:512;
tiny DVE blends (driven by a per-core parity vector) map local <-> global
block order around the collectives.

Latency hiding: scores are built in two passes (own-j columns first, which
only need local qzT, then peer-j columns) so pass 1 overlaps the AllGather
flight; all e-transposes, msg_i matmuls and the global-node tail are deferred
until after the ReduceScatter kickoff so they overlap its flight.  e is
stored in fp8 (raw exp scores are ~e^{+-0.6}, perfectly in fp8e4 range) so
all 8 heads fit in SBUF across the two passes; softmax 1/Z (x a global M=1024
message scale, removed at F_Z assembly) is folded in-place into e.
"""
import sys
import os
import contextlib

if '/opt/trn_rl_repo' not in sys.path:
    sys.path.insert(0, '/opt/trn_rl_repo')

import numpy as np
import concourse.bacc as bacc
import concourse.mybir as mybir
import concourse.tile as tile
from concourse.masks import make_identity

F32 = mybir.dt.float32
F16 = mybir.dt.float16
F8 = mybir.dt.float8e4
U16 = mybir.dt.uint16
AF = mybir.ActivationFunctionType

B = 4
L = 1024
Lh = L // 2          # own token half
D = 128
H = 8
G = 64
IC = L // 128         # 8 j-chunks
ICO = Lh // 128       # 4 own i-chunks
N_CORES = 8
MSC = 1024.0          # global message scale M

_CACHE = {}

DBG_NO_CC = os.environ.get("DBG_NO_CC", "0") == "1"


def build_kernel(n_iter=4, num_devices=8):
    groups = [[2 * i, 2 * i + 1] for i in range(num_devices // 2)]
    nc = bacc.Bacc("TRN2", target_bir_lowering=False, debug=False,
                   num_devices=num_devices)

    xT = nc.declare_dram_parameter("xT", [D, L], F16, isOutput=False)
    tern_a = nc.declare_dram_parameter("tern_a", [D, H * D], F16, isOutput=False)
    tern_b = nc.declare_dram_parameter("tern_b", [D, H * D], F16, isOutput=False)
    glT = nc.declare_dram_parameter("glT", [D, H * G], F16, isOutput=False)
    gl2 = nc.declare_dram_parameter("gl2", [2 * G, (H // 2) * D], F16, isOutput=False)
    pv = nc.declare_dram_parameter("pv", [128, 2], F32, isOutput=False)
    y = nc.declare_dram_parameter("y", [Lh, D], F32, isOutput=True)

    with tile.TileContext(nc) as tc:
        with contextlib.ExitStack() as ctx:
            singles = ctx.enter_context(tc.tile_pool(name="singles", bufs=1))
            sb = ctx.enter_context(tc.tile_pool(name="sb", bufs=2))
            hp = ctx.enter_context(tc.tile_pool(name="hp", bufs=2))
            # PSUM budget (8 banks): fh 1x2 + aux 1x2 + tps 1x2 + msgp 2x1 = 8
            fh = ctx.enter_context(tc.tile_pool(name="fh", bufs=2, space="PSUM"))
            aux = ctx.enter_context(tc.tile_pool(name="aux", bufs=2, space="PSUM"))
            msgp = ctx.enter_context(tc.tile_pool(name="msgp", bufs=1, space="PSUM"))
            dram = ctx.enter_context(tc.tile_pool(name="dram", bufs=2, space="DRAM"))

            # ---- persistent SBUF state ----
            unaryT = singles.tile([D, L], F16)
            nc.sync.dma_start(unaryT[:], xT[:])
            ta_sb = singles.tile([D, H * D], F16)
            nc.sync.dma_start(ta_sb[:], tern_a[:])
            tb_sb = singles.tile([D, H * D], F16)
            nc.sync.dma_start(tb_sb[:], tern_b[:])
            glT_sb = singles.tile([D, H * G], F16)
            nc.sync.dma_start(glT_sb[:], glT[:])
            gl2_sb = singles.tile([2 * G, (H // 2) * D], F16)
            nc.sync.dma_start(gl2_sb[:], gl2[:])
            pv_sb = singles.tile([128, 2], F32)
            nc.sync.dma_start(pv_sb[:], pv[:])
            id16 = singles.tile([128, 128], F16)
            make_identity(nc, id16[:])
            ones128 = singles.tile([128, 128], F16)
            nc.vector.memset(ones128[:], 1.0)
            qzT = singles.tile([D, L], F16)
            # raw exp of own-j scores for all heads: [i128, (h, ic, j_own)] fp16
            e_own = singles.tile([128, H * ICO * Lh], F16)
            # et (normalized transposed probs * M): [j128, (h, jc, i_own)] fp8
            et8 = singles.tile([128, H * IC * 512], F8)
            et83 = et8.rearrange("p (h jc i) -> p h jc i", h=H, jc=IC)
            # r for all j: [j128, (jc, h, a)] fp16
            r_all = singles.tile([128, IC * H * 128], F16)
            r3 = r_all.rearrange("p (jc h a) -> p jc h a", jc=IC, h=H)
            # st (s^T) for all heads: [b128, (h, i_own)] fp16
            st_all = singles.tile([128, H * Lh], F16)
            # per-head softmax scalars
            sums_own = singles.tile([128, H * ICO], F32)
            sums_peer = singles.tile([128, H * ICO], F32)
            rrM = singles.tile([128, H * ICO], F32)

            def init_softmax():
                """qzT <- softmax_D(unary)^T for all 1024 tokens."""
                ez = sb.tile([128, L], F32, tag="ez_init")
                sums = sb.tile([128, IC], F32, tag="zsums_init")
                for c in range(IC):
                    cs = slice(c * 128, (c + 1) * 128)
                    u_ps = aux.tile([128, 128], F16, tag="aux_ps")
                    nc.tensor.transpose(u_ps[:], unaryT[:, cs], id16[:])
                    nc.scalar.activation(ez[:, cs], u_ps[:], AF.Exp,
                                         accum_out=sums[:, c:c + 1])
                rz = sb.tile([128, IC], F32, tag="zrz_init")
                nc.vector.reciprocal(rz[:], sums[:])
                qz_sc = sb.tile([128, L], F16, tag="qzsc_init")
                for c in range(IC):
                    cs = slice(c * 128, (c + 1) * 128)
                    nc.vector.tensor_scalar_mul(qz_sc[:, cs], ez[:, cs], rz[:, c:c + 1])
                for c in range(IC):
                    cs = slice(c * 128, (c + 1) * 128)
                    q_ps = aux.tile([128, 128], F16, tag="aux_ps")
                    nc.tensor.transpose(q_ps[:], qz_sc[:, cs], id16[:])
                    nc.vector.tensor_copy(qzT[:, cs], q_ps[:])

            init_softmax()

            for it in range(n_iter):
                last = (it == n_iter - 1)

                # ---------- AG-independent work (own qz half only) ----------
                # r[j, (h,a)] for own j-chunks
                def build_r(jcs):
                    for jc in jcs:
                        cs = slice(jc * 128, (jc + 1) * 128)
                        r_ps = fh.tile([128, H * 128], F32, tag="fh_ps")
                        for hh in range(2):
                            nc.tensor.matmul(r_ps[:, hh * 512:(hh + 1) * 512],
                                             qzT[:, cs],
                                             tb_sb[:, hh * 512:(hh + 1) * 512])
                        eng = nc.scalar if jc % 2 == 0 else nc.vector
                        if jc % 2 == 0:
                            nc.scalar.copy(
                                r_all[:, jc * H * 128:(jc + 1) * H * 128], r_ps[:])
                        else:
                            nc.vector.tensor_copy(
                                r_all[:, jc * H * 128:(jc + 1) * H * 128], r_ps[:])

                build_r(range(ICO))

                # F_Hg + eg for own i-chunks: eg_sb [i128, (ic, h, g)]
                eg_sb = sb.tile([128, ICO * H * G], F16, tag="eg_sb")
                eg3 = eg_sb.rearrange("p (ic h g) -> p ic h g", ic=ICO, h=H)
                eg_sums = sb.tile([128, ICO * H], F32, tag="eg_sums")
                for ic in range(ICO):
                    cs = slice(ic * 128, (ic + 1) * 128)
                    hg_ps = aux.tile([128, H * G], F32, tag="aux_ps")
                    nc.tensor.matmul(hg_ps[:], qzT[:, cs], glT_sb[:])
                    nc.scalar.activation(eg_sb[:, ic * H * G:(ic + 1) * H * G],
                                         hg_ps[:], AF.Exp)
                nc.vector.reduce_sum(
                    eg_sums.rearrange("p (ic h) -> p ic h", ic=ICO),
                    eg3, axis=mybir.AxisListType.X)

                # pass 1: st, scores-own, exp-own, s for every head
                for h in range(H):
                    hs = slice(h * 128, (h + 1) * 128)
                    st_ps = aux.tile([128, Lh], F32, tag="aux_ps")
                    nc.tensor.matmul(st_ps[:], ta_sb[:, hs], qzT[:, 0:Lh])
                    st_sb = st_all[:, h * Lh:(h + 1) * Lh]
                    nc.scalar.copy(st_sb, st_ps[:])
                    for ic in range(ICO):
                        fo_ps = fh.tile([128, Lh], F32, tag="fh_ps")
                        nc.tensor.matmul(fo_ps[:],
                                         st_all[:, h * Lh + ic * 128:h * Lh + (ic + 1) * 128],
                                         qzT[:, 0:Lh])
                        col = (h * ICO + ic) * Lh
                        nc.scalar.activation(
                            e_own[:, col:col + Lh], fo_ps[:], AF.Exp,
                            accum_out=sums_own[:, h * ICO + ic:h * ICO + ic + 1])

                # ---------- AG-dependent: peer qz half ----------
                # (for it==0 qzT is fully initialized locally)
                build_r(range(ICO, IC))

                # pass 2: scores-peer + exp, then normalize e + msg_j
                msg_ps = msgp.tile([128, L], F32, tag="msg_ps")

                ep_tiles = [None] * H

                def emit_p2_scores(h):
                    e_peer = hp.tile([128, ICO * Lh], F16, tag="e_peer")
                    for ic in range(ICO):
                        fp_ps = fh.tile([128, Lh], F32, tag="fh_ps")
                        nc.tensor.matmul(fp_ps[:],
                                         st_all[:, h * Lh + ic * 128:h * Lh + (ic + 1) * 128],
                                         qzT[:, Lh:L])
                        nc.scalar.activation(
                            e_peer[:, ic * Lh:(ic + 1) * Lh], fp_ps[:], AF.Exp,
                            accum_out=sums_peer[:, h * ICO + ic:h * ICO + ic + 1])
                    ep_tiles[h] = e_peer

                def emit_p2_norm_msgj(h):
                    hs = slice(h * 128, (h + 1) * 128)
                    e_peer = ep_tiles[h]
                    so = sums_own[:, h * ICO:(h + 1) * ICO]
                    sp = sums_peer[:, h * ICO:(h + 1) * ICO]
                    eg_h_sums = eg_sums.rearrange("p (s h) -> p s h", h=H)[:, :, h]
                    tot = hp.tile([128, ICO], F32, tag="htot")
                    nc.vector.tensor_add(tot[:], so, sp)
                    tot2 = hp.tile([128, ICO], F32, tag="htot2")
                    nc.vector.tensor_add(tot2[:], tot[:], eg_h_sums)
                    rr = hp.tile([128, ICO], F32, tag="hr")
                    nc.vector.reciprocal(rr[:], tot2[:])
                    rrM_h = rrM[:, h * ICO:(h + 1) * ICO]
                    nc.vector.tensor_scalar_mul(rrM_h, rr[:], MSC)
                    # e_norm[i, (ic, j)] = Qhs[i, j] * M  (fp16)
                    e_norm = hp.tile([128, ICO * L], F16, tag="e_norm")
                    en3 = e_norm.rearrange("p (ic j) -> p ic j", ic=ICO)
                    for ic in range(ICO):
                        nc.vector.tensor_scalar_mul(
                            en3[:, ic, 0:Lh],
                            e_own[:, (h * ICO + ic) * Lh:(h * ICO + ic + 1) * Lh],
                            rrM_h[:, ic:ic + 1])
                        nc.vector.tensor_scalar_mul(
                            en3[:, ic, Lh:L],
                            e_peer[:, ic * Lh:(ic + 1) * Lh],
                            rrM_h[:, ic:ic + 1])
                    # s (raw) for this head
                    s_ps = aux.tile([128, ICO * 128], F32, tag="aux_ps")
                    for ic in range(ICO):
                        nc.tensor.matmul(s_ps[:, ic * 128:(ic + 1) * 128],
                                         qzT[:, ic * 128:(ic + 1) * 128],
                                         ta_sb[:, hs])
                    s_raw = hp.tile([128, ICO * 128], F16, tag="s_raw")
                    nc.scalar.copy(s_raw[:], s_ps[:])
                    # msg_j: out [b, j] += sum_i s_raw[i,b] * e_norm[i,j]
                    for ic in range(ICO):
                        for half in range(2):
                            nc.tensor.matmul(
                                msg_ps[:, half * 512:(half + 1) * 512],
                                s_raw[:, ic * 128:(ic + 1) * 128],
                                en3[:, ic, half * 512:(half + 1) * 512],
                                start=(h == 0 and ic == 0),
                                stop=(h == H - 1 and ic == ICO - 1),
                                skip_group_check=True)
                    # transposes of e_norm: et[h][jc][:, i] = Qhs[i, j]*M (fp8 out)
                    for jc in range(IC):
                        t_ps = aux.tile([128, 512], F16, tag="aux_ps")
                        for ic in range(ICO):
                            nc.tensor.transpose(
                                t_ps[:, ic * 128:(ic + 1) * 128],
                                en3[:, ic, jc * 128:(jc + 1) * 128],
                                id16[:])
                        dst = et8[:, (h * IC + jc) * 512:(h * IC + jc + 1) * 512]
                        if jc % 2 == 0:
                            nc.vector.tensor_copy(dst, t_ps[:])
                        else:
                            nc.scalar.copy(dst, t_ps[:])

                emit_p2_scores(0)
                for h in range(H):
                    if h + 1 < H:
                        emit_p2_scores(h + 1)
                    emit_p2_norm_msgj(h)

                # ---------- RS kickoff ----------
                bi0 = sb.tile([128, Lh], F16, tag="bi0")
                bi1 = sb.tile([128, Lh], F16, tag="bi1")
                ta0 = sb.tile([128, Lh], F16, tag="ta0")
                ta1 = sb.tile([128, Lh], F16, tag="ta1")
                tb0 = sb.tile([128, Lh], F16, tag="tb0")
                tb1 = sb.tile([128, Lh], F16, tag="tb1")
                # block0 = own*(1-p) + peer*p ; block1 = own*p + peer*(1-p)
                nc.vector.tensor_scalar_mul(ta0[:], msg_ps[:, 0:Lh], pv_sb[:, 1:2])
                nc.vector.tensor_scalar_mul(tb0[:], msg_ps[:, Lh:L], pv_sb[:, 0:1])
                nc.vector.tensor_add(bi0[:], ta0[:], tb0[:])
                nc.vector.tensor_scalar_mul(ta1[:], msg_ps[:, 0:Lh], pv_sb[:, 0:1])
                nc.vector.tensor_scalar_mul(tb1[:], msg_ps[:, Lh:L], pv_sb[:, 1:2])
                nc.vector.tensor_add(bi1[:], ta1[:], tb1[:])
                rs_in = dram.tile([2 * 128, Lh], F16, tag="rs_in")
                rs_out = dram.tile([128, Lh], F16, tag="rs_out")
                nc.sync.dma_start(rs_in[0:128, :], bi0[:])
                nc.scalar.dma_start(rs_in[128:256, :], bi1[:])
                if DBG_NO_CC:
                    nc.sync.dma_start(rs_out[:], rs_in[0:128, :])
                else:
                    nc.gpsimd.collective_compute(
                        "ReduceScatter", mybir.AluOpType.add,
                        replica_groups=groups,
                        ins=[rs_in.opt()], outs=[rs_out.opt()])

                # ---------- deferred (overlaps RS): msg_i, eg ----------
                for h in range(H):
                    # msg_i: out [a, i_own] += sum_j r[j,a] * QhsT[j,i] * M
                    for jc in range(IC):
                        nc.tensor.matmul(
                            msg_ps[:, 0:Lh],
                            r3[:, jc, h, :],
                            et83[:, h, jc, :],
                            start=(h == 0 and jc == 0), stop=False,
                            skip_group_check=True)

                # eg tail: normalize, pair-stacked transposes, matmuls
                egn_sb = sb.tile([128, ICO * H * G], F16, tag="egn_sb")
                egn3 = egn_sb.rearrange("p (ic h g) -> p ic h g", ic=ICO, h=H)
                for ic in range(ICO):
                    for h in range(H):
                        nc.vector.tensor_scalar_mul(
                            egn3[:, ic, h, :], eg3[:, ic, h, :],
                            rrM[:, h * ICO + ic:h * ICO + ic + 1])
                for k in range(H // 2):
                    egT_ps = aux.tile([128, ICO * 128], F16, tag="aux_ps")
                    for ic in range(ICO):
                        nc.tensor.transpose(
                            egT_ps[:, ic * 128:(ic + 1) * 128],
                            egn_sb[:, (ic * H + 2 * k) * G:(ic * H + 2 * k) * G + 2 * G],
                            id16[:])
                    egT_sb = hp.tile([128, ICO * 128], F16, tag="egT_sb")
                    nc.scalar.copy(egT_sb[:], egT_ps[:])
                    nc.tensor.matmul(msg_ps[:, 0:Lh],
                                     gl2_sb[:, k * 128:(k + 1) * 128],
                                     egT_sb[:],
                                     start=False, stop=(k == H // 2 - 1),
                                     skip_group_check=True)

                # ---------- RS result -> F_Z own -> z tail ----------
                msg_red = sb.tile([128, Lh], F16, tag="msg_red")
                nc.sync.dma_start(msg_red[:], rs_out[:])
                t1 = sb.tile([128, Lh], F32, tag="t1")
                nc.vector.tensor_add(t1[:], msg_ps[:, 0:Lh], msg_red[:])
                t2 = sb.tile([128, Lh], F16, tag="t2")
                nc.vector.tensor_scalar_mul(t2[:], t1[:], 1.0 / MSC)
                fz16 = sb.tile([128, Lh], F16, tag="fz16")
                nc.vector.tensor_add(fz16[:], t2[:], unaryT[:, 0:Lh])

                if last:
                    out_sb = sb.tile([128, Lh], F32, tag="zout")
                    for ic in range(ICO):
                        cs = slice(ic * 128, (ic + 1) * 128)
                        fz_ps = aux.tile([128, 128], F16, tag="aux_ps")
                        nc.tensor.transpose(fz_ps[:], fz16[:, cs], id16[:])
                        nc.scalar.copy(out_sb[:, cs], fz_ps[:])
                        nc.sync.dma_start(y[ic * 128:(ic + 1) * 128, :],
                                          out_sb[:, cs])
                else:
                    ez = sb.tile([128, Lh], F32, tag="ez")
                    zsums = sb.tile([128, ICO], F32, tag="zsums")
                    for ic in range(ICO):
                        cs = slice(ic * 128, (ic + 1) * 128)
                        fz_ps = aux.tile([128, 128], F16, tag="aux_ps")
                        nc.tensor.transpose(fz_ps[:], fz16[:, cs], id16[:])
                        nc.scalar.activation(ez[:, cs], fz_ps[:], AF.Exp,
                                             accum_out=zsums[:, ic:ic + 1])
                    rz = sb.tile([128, ICO], F32, tag="zrz")
                    nc.vector.reciprocal(rz[:], zsums[:])
                    qz_sc = sb.tile([128, Lh], F16, tag="qzsc")
                    for ic in range(ICO):
                        cs = slice(ic * 128, (ic + 1) * 128)
                        nc.vector.tensor_scalar_mul(qz_sc[:, cs], ez[:, cs],
                                                    rz[:, ic:ic + 1])
                    qzT_ps = aux.tile([128, Lh], F16, tag="aux_ps")
                    for ic in range(ICO):
                        cs = slice(ic * 128, (ic + 1) * 128)
                        nc.tensor.transpose(qzT_ps[:, cs], qz_sc[:, cs], id16[:])
                    nc.vector.tensor_copy(qzT[:, 0:Lh], qzT_ps[:])

                    # ---------- AG of own qz half ----------
                    ag_in = dram.tile([128, Lh], F16, tag="ag_in")
                    ag_out = dram.tile([2 * 128, Lh], F16, tag="ag_out")
                    nc.sync.dma_start(ag_in[:], qzT[:, 0:Lh])
                    if DBG_NO_CC:
                        nc.sync.dma_start(ag_out[0:128, :], ag_in[:])
                        nc.sync.dma_start(ag_out[128:256, :], ag_in[:])
                    else:
                        nc.gpsimd.collective_compute(
                            "AllGather", mybir.AluOpType.bypass,
                            replica_groups=groups,
                            ins=[ag_in.opt()], outs=[ag_out.opt()])
                    g0 = sb.tile([128, Lh], F16, tag="g0")
                    g1 = sb.tile([128, Lh], F16, tag="g1")
                    nc.sync.dma_start(g0[:], ag_out[0:128, :])
                    nc.scalar.dma_start(g1[:], ag_out[128:256, :])
                    # peer half = g0*p + g1*(1-p)
                    gp = sb.tile([128, Lh], F16, tag="gp")
                    gq = sb.tile([128, Lh], F16, tag="gq")
                    nc.vector.tensor_scalar_mul(gp[:], g0[:], pv_sb[:, 0:1])
                    nc.vector.tensor_scalar_mul(gq[:], g1[:], pv_sb[:, 1:2])
                    nc.vector.tensor_add(qzT[:, Lh:L], gp[:], gq[:])

    nc.compile()
    return nc


class _Runner:
    """Keeps the jitted SPMD executable alive across kernel() calls."""

    def __init__(self, nc):
        import jax
        from jax.sharding import Mesh, PartitionSpec
        from jax.experimental.shard_map import shard_map
        from concourse.bass2jax import (_bass_exec_p, install_neuronx_cc_hook,
                                        partition_id_tensor)
        install_neuronx_cc_hook()
        self.jax = jax
        in_names, out_names, out_avals, zero_outs = [], [], [], []
        partition_name = nc.partition_id_tensor.name if nc.partition_id_tensor else None
        for alloc in nc.m.functions[0].allocations:
            if not isinstance(alloc, mybir.MemoryLocationSet):
                continue
            name = alloc.memorylocations[0].name
            if alloc.kind == "ExternalInput":
                if name != partition_name:
                    in_names.append(name)
            elif alloc.kind == "ExternalOutput":
                out_names.append(name)
                shape = tuple(alloc.tensor_shape)
                dtype = mybir.dt.np(alloc.dtype)
                out_avals.append(jax.core.ShapedArray(shape, dtype))
                zero_outs.append(np.zeros(shape, dtype))
        self.in_names, self.out_names = in_names, out_names
        self.out_avals, self.zero_outs = out_avals, zero_outs
        all_in_names = list(in_names) + list(out_names)
        if partition_name is not None:
            all_in_names.append(partition_name)

        def _body(*args):
            operands = list(args)
            if partition_name is not None:
                operands.append(partition_id_tensor())
            outs = _bass_exec_p.bind(
                *operands,
                out_avals=tuple(out_avals),
                in_names=tuple(all_in_names),
                out_names=tuple(out_names),
                lowering_input_output_aliases=(),
                sim_require_finite=True,
                sim_require_nnan=True,
                nc=nc,
            )
            return tuple(outs)

        devices = jax.devices()[:N_CORES]
        mesh = Mesh(np.asarray(devices), ("core",))
        n_params = len(in_names)
        in_specs = (PartitionSpec("core"),) * (n_params + len(out_names))
        out_specs = (PartitionSpec("core"),) * len(out_names)
        self.fn = jax.jit(shard_map(_body, mesh=mesh, in_specs=in_specs,
                                    out_specs=out_specs, check_rep=False),
                          keep_unused=True)

    def __call__(self, in_maps):
        jax = self.jax
        concat_in = [
            np.concatenate([np.asarray(in_maps[c][name]) for c in range(N_CORES)], axis=0)
            for name in self.in_names
        ]
        concat_zeros = [np.zeros((N_CORES * z.shape[0], *z.shape[1:]), z.dtype)
                        for z in self.zero_outs]
        outs = self.fn(*concat_in, *concat_zeros)
        jax.block_until_ready(outs)
        return [
            {name: np.asarray(outs[i]).reshape(N_CORES, *self.out_avals[i].shape)[c]
             for i, name in enumerate(self.out_names)}
            for c in range(N_CORES)
        ]


def make_core_inputs(x, ternary, global_, core):
    n, half = core // 2, core % 2
    t = ternary
    g = global_
    perm = np.concatenate([np.arange(half * Lh, (half + 1) * Lh),
                           np.arange((1 - half) * Lh, (2 - half) * Lh)])
    xT = np.ascontiguousarray(x[n].T[:, perm].astype(np.float16))
    pvv = np.zeros((128, 2), np.float32)
    pvv[:, 0] = float(half)
    pvv[:, 1] = 1.0 - float(half)
    return {
        "xT": xT,
        "tern_a": np.ascontiguousarray(t.transpose(0, 2, 1).reshape(D, H * D).astype(np.float16)),
        "tern_b": np.ascontiguousarray(t.transpose(1, 2, 0).reshape(D, H * D).astype(np.float16)),
        "glT": np.ascontiguousarray(g.transpose(1, 2, 0).reshape(D, H * G).astype(np.float16)),
        "gl2": np.ascontiguousarray(
            g.transpose(2, 0, 1).reshape(H // 2, 2, G, D)
             .transpose(1, 2, 0, 3).reshape(2 * G, (H // 2) * D).astype(np.float16)),
        "pv": pvv,
    }


def get_runner(n_iter=4):
    key = ("runner", n_iter)
    if key not in _CACHE:
        nc = build_kernel(n_iter=n_iter, num_devices=N_CORES)
        _CACHE[key] = _Runner(nc)
    return _CACHE[key]


def kernel(x, mask, ternary, global_):
    x = np.asarray(x, dtype=np.float32)
    mask = np.asarray(mask)
    ternary = np.asarray(ternary, dtype=np.float32)
    global_ = np.asarray(global_, dtype=np.float32)

    run = get_runner(4)
    in_maps = [make_core_inputs(x, ternary, global_, c) for c in range(N_CORES)]
    res = run(in_maps)
    out = np.stack([np.concatenate([res[2 * n]["y"], res[2 * n + 1]["y"]], axis=0)
                    for n in range(B)])
    out = np.where((mask != 0)[..., None], out, np.float32(0.0)).astype(np.float32)
    return out


# revision 3
# speedup vs baseline: 1.4561x; 1.2033x over previous
"""Trainium2 Bass kernel v2 for nn_AbsGlobalHeadProbEncoder (MFVI message passing).

Sequence-parallel over the query (i) axis: 8 cores = 4 batch elements x 2
token-halves.  Each core computes head scores / messages only for its own 512
query rows (all 8 heads), so per-core PE work halves vs the replicated
baseline.  Cross-core per iteration:
  - ReduceScatter (fp16) of the partial msg_j [D, L] -> each core gets the
    fully-summed messages for its own token half.
  - AllGather (fp16) of the updated Qz^T half -> full qzT for the next
    iteration's scores.
Inputs are host-permuted so each core's own tokens are local columns 0:512;
tiny DVE blends (driven by a per-core parity vector) map local <-> global
block order around the collectives.

Latency hiding: scores are built in two passes (own-j columns first, which
only need local qzT, then peer-j columns) so pass 1 overlaps the AllGather
flight; all e-transposes, msg_i matmuls and the global-node tail are deferred
until after the ReduceScatter kickoff so they overlap its flight.  e is
stored in fp8 (raw exp scores are ~e^{+-0.6}, perfectly in fp8e4 range) so
all 8 heads fit in SBUF across the two passes; softmax 1/Z (x a global M=1024
message scale, removed at F_Z assembly) is folded in-place into e.
"""
import sys
import os
import contextlib

if '/opt/trn_rl_repo' not in sys.path:
    sys.path.insert(0, '/opt/trn_rl_repo')

import numpy as np
import concourse.bacc as bacc
import concourse.mybir as mybir
import concourse.tile as tile
from concourse.masks import make_identity

F32 = mybir.dt.float32
F16 = mybir.dt.float16
F8 = mybir.dt.float8e4
U16 = mybir.dt.uint16
AF = mybir.ActivationFunctionType

B = 4
L = 1024
Lh = L // 2          # own token half
D = 128
H = 8
G = 64
IC = L // 128         # 8 j-chunks
ICO = Lh // 128       # 4 own i-chunks
N_CORES = 8
MSC = 1024.0          # global message scale M

_CACHE = {}

DBG_NO_CC = os.environ.get("DBG_NO_CC", "0") == "1"


def build_kernel(n_iter=4, num_devices=8):
    groups = [[2 * i, 2 * i + 1] for i in range(num_devices // 2)]
    nc = bacc.Bacc("TRN2", target_bir_lowering=False, debug=False,
                   num_devices=num_devices)

    xT = nc.declare_dram_parameter("xT", [D, L], F16, isOutput=False)
    tern_a = nc.declare_dram_parameter("tern_a", [D, H * D], F16, isOutput=False)
    tern_b = nc.declare_dram_parameter("tern_b", [D, H * D], F16, isOutput=False)
    glT = nc.declare_dram_parameter("glT", [D, H * G], F16, isOutput=False)
    gl2 = nc.declare_dram_parameter("gl2", [2 * G, (H // 2) * D], F16, isOutput=False)
    pv = nc.declare_dram_parameter("pv", [128, 2], F32, isOutput=False)
    y = nc.declare_dram_parameter("y", [Lh, D], F32, isOutput=True)

    with tile.TileContext(nc) as tc:
        with contextlib.ExitStack() as ctx:
            singles = ctx.enter_context(tc.tile_pool(name="singles", bufs=1))
            sb = ctx.enter_context(tc.tile_pool(name="sb", bufs=2))
            hp = ctx.enter_context(tc.tile_pool(name="hp", bufs=3))
            # PSUM budget (8 banks): fh 1x2 + aux 1x2 + tps 1x2 + msgp 2x1 = 8
            fh = ctx.enter_context(tc.tile_pool(name="fh", bufs=2, space="PSUM"))
            aux = ctx.enter_context(tc.tile_pool(name="aux", bufs=2, space="PSUM"))
            msgp = ctx.enter_context(tc.tile_pool(name="msgp", bufs=1, space="PSUM"))
            dram = ctx.enter_context(tc.tile_pool(name="dram", bufs=2, space="DRAM"))

            # ---- persistent SBUF state ----
            unaryT = singles.tile([D, L], F16)
            nc.sync.dma_start(unaryT[:], xT[:])
            ta_sb = singles.tile([D, H * D], F16)
            nc.sync.dma_start(ta_sb[:], tern_a[:])
            tb_sb = singles.tile([D, H * D], F16)
            nc.sync.dma_start(tb_sb[:], tern_b[:])
            glT_sb = singles.tile([D, H * G], F16)
            nc.sync.dma_start(glT_sb[:], glT[:])
            gl2_sb = singles.tile([2 * G, (H // 2) * D], F16)
            nc.sync.dma_start(gl2_sb[:], gl2[:])
            pv_sb = singles.tile([128, 2], F32)
            nc.sync.dma_start(pv_sb[:], pv[:])
            id16 = singles.tile([128, 128], F16)
            make_identity(nc, id16[:])
            ones128 = singles.tile([128, 128], F16)
            nc.vector.memset(ones128[:], 1.0)
            qzT = singles.tile([D, L], F16)
            # raw exp of own-j scores for all heads: [i128, (h, ic, j_own)] fp16
            e_own = singles.tile([128, H * ICO * Lh], F16)
            # et (normalized transposed probs * M): [j128, (h, jc, i_own)] fp8
            et8 = singles.tile([128, H * IC * 512], F8)
            et83 = et8.rearrange("p (h jc i) -> p h jc i", h=H, jc=IC)
            # r for all j: [j128, (jc, h, a)] fp16
            r_all = singles.tile([128, IC * H * 128], F16)
            r3 = r_all.rearrange("p (jc h a) -> p jc h a", jc=IC, h=H)
            # st (s^T) for all heads: [b128, (h, i_own)] fp16
            st_all = singles.tile([128, H * Lh], F16)
            # per-head softmax scalars
            sums_own = singles.tile([128, H * ICO], F32)
            sums_peer = singles.tile([128, H * ICO], F32)
            rrM = singles.tile([128, H * ICO], F32)

            def init_softmax():
                """qzT <- softmax_D(unary)^T for all 1024 tokens."""
                ez = sb.tile([128, L], F32, tag="ez_init")
                sums = sb.tile([128, IC], F32, tag="zsums_init")
                for c in range(IC):
                    cs = slice(c * 128, (c + 1) * 128)
                    u_ps = aux.tile([128, 128], F16, tag="aux_ps")
                    nc.tensor.transpose(u_ps[:], unaryT[:, cs], id16[:])
                    nc.scalar.activation(ez[:, cs], u_ps[:], AF.Exp,
                                         accum_out=sums[:, c:c + 1])
                rz = sb.tile([128, IC], F32, tag="zrz_init")
                nc.vector.reciprocal(rz[:], sums[:])
                qz_sc = sb.tile([128, L], F16, tag="qzsc_init")
                for c in range(IC):
                    cs = slice(c * 128, (c + 1) * 128)
                    nc.vector.tensor_scalar_mul(qz_sc[:, cs], ez[:, cs], rz[:, c:c + 1])
                for c in range(IC):
                    cs = slice(c * 128, (c + 1) * 128)
                    q_ps = aux.tile([128, 128], F16, tag="aux_ps")
                    nc.tensor.transpose(q_ps[:], qz_sc[:, cs], id16[:])
                    nc.vector.tensor_copy(qzT[:, cs], q_ps[:])

            init_softmax()

            for it in range(n_iter):
                last = (it == n_iter - 1)

                # ---------- AG-independent work (own qz half only) ----------
                # r[j, (h,a)] for own j-chunks
                def build_r(jcs):
                    for jc in jcs:
                        cs = slice(jc * 128, (jc + 1) * 128)
                        r_ps = fh.tile([128, H * 128], F32, tag="fh_ps")
                        for hh in range(2):
                            nc.tensor.matmul(r_ps[:, hh * 512:(hh + 1) * 512],
                                             qzT[:, cs],
                                             tb_sb[:, hh * 512:(hh + 1) * 512])
                        eng = nc.scalar if jc % 2 == 0 else nc.vector
                        if jc % 2 == 0:
                            nc.scalar.copy(
                                r_all[:, jc * H * 128:(jc + 1) * H * 128], r_ps[:])
                        else:
                            nc.vector.tensor_copy(
                                r_all[:, jc * H * 128:(jc + 1) * H * 128], r_ps[:])

                build_r(range(ICO))

                # F_Hg + eg for own i-chunks: eg_sb [i128, (ic, h, g)]
                eg_sb = sb.tile([128, ICO * H * G], F16, tag="eg_sb")
                eg3 = eg_sb.rearrange("p (ic h g) -> p ic h g", ic=ICO, h=H)
                eg_sums = sb.tile([128, ICO * H], F32, tag="eg_sums")
                for ic in range(ICO):
                    cs = slice(ic * 128, (ic + 1) * 128)
                    hg_ps = aux.tile([128, H * G], F32, tag="aux_ps")
                    nc.tensor.matmul(hg_ps[:], qzT[:, cs], glT_sb[:])
                    nc.scalar.activation(eg_sb[:, ic * H * G:(ic + 1) * H * G],
                                         hg_ps[:], AF.Exp)
                nc.vector.reduce_sum(
                    eg_sums.rearrange("p (ic h) -> p ic h", ic=ICO),
                    eg3, axis=mybir.AxisListType.X)

                # pass 1: st, scores-own, exp-own, s for every head
                for h in range(H):
                    hs = slice(h * 128, (h + 1) * 128)
                    st_ps = aux.tile([128, Lh], F32, tag="aux_ps")
                    nc.tensor.matmul(st_ps[:], ta_sb[:, hs], qzT[:, 0:Lh])
                    st_sb = st_all[:, h * Lh:(h + 1) * Lh]
                    nc.scalar.copy(st_sb, st_ps[:])
                    for ic in range(ICO):
                        fo_ps = fh.tile([128, Lh], F32, tag="fh_ps")
                        nc.tensor.matmul(fo_ps[:],
                                         st_all[:, h * Lh + ic * 128:h * Lh + (ic + 1) * 128],
                                         qzT[:, 0:Lh])
                        col = (h * ICO + ic) * Lh
                        nc.scalar.activation(
                            e_own[:, col:col + Lh], fo_ps[:], AF.Exp,
                            accum_out=sums_own[:, h * ICO + ic:h * ICO + ic + 1])

                # ---------- AG-dependent: peer qz half ----------
                # (for it==0 qzT is fully initialized locally)
                build_r(range(ICO, IC))

                # pass 2: scores-peer + exp, then normalize e + msg_j
                msg_ps = msgp.tile([128, L], F32, tag="msg_ps")

                ep_tiles = [None] * H

                def emit_p2_scores(h):
                    e_peer = hp.tile([128, ICO * Lh], F16, tag="e_peer")
                    for ic in range(ICO):
                        fp_ps = fh.tile([128, Lh], F32, tag="fh_ps")
                        nc.tensor.matmul(fp_ps[:],
                                         st_all[:, h * Lh + ic * 128:h * Lh + (ic + 1) * 128],
                                         qzT[:, Lh:L])
                        nc.scalar.activation(
                            e_peer[:, ic * Lh:(ic + 1) * Lh], fp_ps[:], AF.Exp,
                            accum_out=sums_peer[:, h * ICO + ic:h * ICO + ic + 1])
                    ep_tiles[h] = e_peer

                def emit_p2_norm_msgj(h):
                    hs = slice(h * 128, (h + 1) * 128)
                    e_peer = ep_tiles[h]
                    so = sums_own[:, h * ICO:(h + 1) * ICO]
                    sp = sums_peer[:, h * ICO:(h + 1) * ICO]
                    eg_h_sums = eg_sums.rearrange("p (s h) -> p s h", h=H)[:, :, h]
                    tot = hp.tile([128, ICO], F32, tag="htot")
                    nc.vector.tensor_add(tot[:], so, sp)
                    tot2 = hp.tile([128, ICO], F32, tag="htot2")
                    nc.vector.tensor_add(tot2[:], tot[:], eg_h_sums)
                    rr = hp.tile([128, ICO], F32, tag="hr")
                    nc.vector.reciprocal(rr[:], tot2[:])
                    rrM_h = rrM[:, h * ICO:(h + 1) * ICO]
                    nc.vector.tensor_scalar_mul(rrM_h, rr[:], MSC)
                    # e_norm[i, (ic, j)] = Qhs[i, j] * M  (fp16)
                    e_norm = hp.tile([128, ICO * L], F16, tag="e_norm")
                    en3 = e_norm.rearrange("p (ic j) -> p ic j", ic=ICO)
                    for ic in range(ICO):
                        nc.vector.tensor_scalar_mul(
                            en3[:, ic, 0:Lh],
                            e_own[:, (h * ICO + ic) * Lh:(h * ICO + ic + 1) * Lh],
                            rrM_h[:, ic:ic + 1])
                        nc.vector.tensor_scalar_mul(
                            en3[:, ic, Lh:L],
                            e_peer[:, ic * Lh:(ic + 1) * Lh],
                            rrM_h[:, ic:ic + 1])
                    # s (raw) for this head
                    s_ps = aux.tile([128, ICO * 128], F32, tag="aux_ps")
                    for ic in range(ICO):
                        nc.tensor.matmul(s_ps[:, ic * 128:(ic + 1) * 128],
                                         qzT[:, ic * 128:(ic + 1) * 128],
                                         ta_sb[:, hs])
                    s_raw = hp.tile([128, ICO * 128], F16, tag="s_raw")
                    nc.scalar.copy(s_raw[:], s_ps[:])
                    # msg_j: out [b, j] += sum_i s_raw[i,b] * e_norm[i,j]
                    for ic in range(ICO):
                        for half in range(2):
                            nc.tensor.matmul(
                                msg_ps[:, half * 512:(half + 1) * 512],
                                s_raw[:, ic * 128:(ic + 1) * 128],
                                en3[:, ic, half * 512:(half + 1) * 512],
                                start=(h == 0 and ic == 0),
                                stop=(h == H - 1 and ic == ICO - 1),
                                skip_group_check=True)
                    # transposes of e_norm: et[h][jc][:, i] = Qhs[i, j]*M (fp8 out)
                    for jc in range(IC):
                        t_ps = aux.tile([128, 512], F16, tag="aux_ps")
                        for ic in range(ICO):
                            nc.tensor.transpose(
                                t_ps[:, ic * 128:(ic + 1) * 128],
                                en3[:, ic, jc * 128:(jc + 1) * 128],
                                id16[:])
                        dst = et8[:, (h * IC + jc) * 512:(h * IC + jc + 1) * 512]
                        if jc % 2 == 0:
                            nc.vector.tensor_copy(dst, t_ps[:])
                        else:
                            nc.scalar.copy(dst, t_ps[:])

                emit_p2_scores(0)
                for h in range(H):
                    if h + 1 < H:
                        emit_p2_scores(h + 1)
                    emit_p2_norm_msgj(h)

                # ---------- RS kickoff ----------
                bi0 = sb.tile([128, Lh], F16, tag="bi0")
                bi1 = sb.tile([128, Lh], F16, tag="bi1")
                ta0 = sb.tile([128, Lh], F16, tag="ta0")
                ta1 = sb.tile([128, Lh], F16, tag="ta1")
                tb0 = sb.tile([128, Lh], F16, tag="tb0")
                tb1 = sb.tile([128, Lh], F16, tag="tb1")
                # block0 = own*(1-p) + peer*p ; block1 = own*p + peer*(1-p)
                nc.vector.tensor_scalar_mul(ta0[:], msg_ps[:, 0:Lh], pv_sb[:, 1:2])
                nc.vector.tensor_scalar_mul(tb0[:], msg_ps[:, Lh:L], pv_sb[:, 0:1])
                nc.vector.tensor_add(bi0[:], ta0[:], tb0[:])
                nc.vector.tensor_scalar_mul(ta1[:], msg_ps[:, 0:Lh], pv_sb[:, 0:1])
                nc.vector.tensor_scalar_mul(tb1[:], msg_ps[:, Lh:L], pv_sb[:, 1:2])
                nc.vector.tensor_add(bi1[:], ta1[:], tb1[:])
                rs_in = dram.tile([2 * 128, Lh], F16, tag="rs_in")
                rs_out = dram.tile([128, Lh], F16, tag="rs_out")
                nc.sync.dma_start(rs_in[0:128, :], bi0[:])
                nc.scalar.dma_start(rs_in[128:256, :], bi1[:])
                if DBG_NO_CC:
                    nc.sync.dma_start(rs_out[:], rs_in[0:128, :])
                else:
                    nc.gpsimd.collective_compute(
                        "ReduceScatter", mybir.AluOpType.add,
                        replica_groups=groups,
                        ins=[rs_in.opt()], outs=[rs_out.opt()])

                # ---------- deferred (overlaps RS): msg_i, eg ----------
                for h in range(H):
                    # msg_i: out [a, i_own] += sum_j r[j,a] * QhsT[j,i] * M
                    for jc in range(IC):
                        nc.tensor.matmul(
                            msg_ps[:, 0:Lh],
                            r3[:, jc, h, :],
                            et83[:, h, jc, :],
                            start=(h == 0 and jc == 0), stop=False,
                            skip_group_check=True)

                # eg tail: normalize, pair-stacked transposes, matmuls
                egn_sb = sb.tile([128, ICO * H * G], F16, tag="egn_sb")
                egn3 = egn_sb.rearrange("p (ic h g) -> p ic h g", ic=ICO, h=H)
                for ic in range(ICO):
                    for h in range(H):
                        nc.vector.tensor_scalar_mul(
                            egn3[:, ic, h, :], eg3[:, ic, h, :],
                            rrM[:, h * ICO + ic:h * ICO + ic + 1])
                for k in range(H // 2):
                    egT_ps = aux.tile([128, ICO * 128], F16, tag="aux_ps")
                    for ic in range(ICO):
                        nc.tensor.transpose(
                            egT_ps[:, ic * 128:(ic + 1) * 128],
                            egn_sb[:, (ic * H + 2 * k) * G:(ic * H + 2 * k) * G + 2 * G],
                            id16[:])
                    egT_sb = hp.tile([128, ICO * 128], F16, tag="egT_sb")
                    nc.scalar.copy(egT_sb[:], egT_ps[:])
                    nc.tensor.matmul(msg_ps[:, 0:Lh],
                                     gl2_sb[:, k * 128:(k + 1) * 128],
                                     egT_sb[:],
                                     start=False, stop=(k == H // 2 - 1),
                                     skip_group_check=True)

                # ---------- RS result -> F_Z own -> z tail ----------
                msg_red = sb.tile([128, Lh], F16, tag="msg_red")
                nc.sync.dma_start(msg_red[:], rs_out[:])
                t1 = sb.tile([128, Lh], F32, tag="t1")
                nc.vector.tensor_add(t1[:], msg_ps[:, 0:Lh], msg_red[:])
                t2 = sb.tile([128, Lh], F16, tag="t2")
                nc.vector.tensor_scalar_mul(t2[:], t1[:], 1.0 / MSC)
                fz16 = sb.tile([128, Lh], F16, tag="fz16")
                nc.vector.tensor_add(fz16[:], t2[:], unaryT[:, 0:Lh])

                if last:
                    out_sb = sb.tile([128, Lh], F32, tag="zout")
                    for ic in range(ICO):
                        cs = slice(ic * 128, (ic + 1) * 128)
                        fz_ps = aux.tile([128, 128], F16, tag="aux_ps")
                        nc.tensor.transpose(fz_ps[:], fz16[:, cs], id16[:])
                        nc.scalar.copy(out_sb[:, cs], fz_ps[:])
                        nc.sync.dma_start(y[ic * 128:(ic + 1) * 128, :],
                                          out_sb[:, cs])
                else:
                    ez = sb.tile([128, Lh], F32, tag="ez")
                    zsums = sb.tile([128, ICO], F32, tag="zsums")
                    for ic in range(ICO):
                        cs = slice(ic * 128, (ic + 1) * 128)
                        fz_ps = aux.tile([128, 128], F16, tag="aux_ps")
                        nc.tensor.transpose(fz_ps[:], fz16[:, cs], id16[:])
                        nc.scalar.activation(ez[:, cs], fz_ps[:], AF.Exp,
                                             accum_out=zsums[:, ic:ic + 1])
                    rz = sb.tile([128, ICO], F32, tag="zrz")
                    nc.vector.reciprocal(rz[:], zsums[:])
                    qz_sc = sb.tile([128, Lh], F16, tag="qzsc")
                    for ic in range(ICO):
                        cs = slice(ic * 128, (ic + 1) * 128)
                        nc.vector.tensor_scalar_mul(qz_sc[:, cs], ez[:, cs],
                                                    rz[:, ic:ic + 1])
                    qzT_ps = aux.tile([128, Lh], F16, tag="aux_ps")
                    for ic in range(ICO):
                        cs = slice(ic * 128, (ic + 1) * 128)
                        nc.tensor.transpose(qzT_ps[:, cs], qz_sc[:, cs], id16[:])
                    nc.vector.tensor_copy(qzT[:, 0:Lh], qzT_ps[:])

                    # ---------- AG of own qz half ----------
                    ag_in = dram.tile([128, Lh], F16, tag="ag_in")
                    ag_out = dram.tile([2 * 128, Lh], F16, tag="ag_out")
                    nc.sync.dma_start(ag_in[:], qzT[:, 0:Lh])
                    if DBG_NO_CC:
                        nc.sync.dma_start(ag_out[0:128, :], ag_in[:])
                        nc.sync.dma_start(ag_out[128:256, :], ag_in[:])
                    else:
                        nc.gpsimd.collective_compute(
                            "AllGather", mybir.AluOpType.bypass,
                            replica_groups=groups,
                            ins=[ag_in.opt()], outs=[ag_out.opt()])
                    g0 = sb.tile([128, Lh], F16, tag="g0")
                    g1 = sb.tile([128, Lh], F16, tag="g1")
                    nc.sync.dma_start(g0[:], ag_out[0:128, :])
                    nc.scalar.dma_start(g1[:], ag_out[128:256, :])
                    # peer half = g0*p + g1*(1-p)
                    gp = sb.tile([128, Lh], F16, tag="gp")
                    gq = sb.tile([128, Lh], F16, tag="gq")
                    nc.vector.tensor_scalar_mul(gp[:], g0[:], pv_sb[:, 0:1])
                    nc.vector.tensor_scalar_mul(gq[:], g1[:], pv_sb[:, 1:2])
                    nc.vector.tensor_add(qzT[:, Lh:L], gp[:], gq[:])

    nc.compile()
    return nc


class _Runner:
    """Keeps the jitted SPMD executable alive across kernel() calls."""

    def __init__(self, nc):
        import jax
        from jax.sharding import Mesh, PartitionSpec
        from jax.experimental.shard_map import shard_map
        from concourse.bass2jax import (_bass_exec_p, install_neuronx_cc_hook,
                                        partition_id_tensor)
        install_neuronx_cc_hook()
        self.jax = jax
        in_names, out_names, out_avals, zero_outs = [], [], [], []
        partition_name = nc.partition_id_tensor.name if nc.partition_id_tensor else None
        for alloc in nc.m.functions[0].allocations:
            if not isinstance(alloc, mybir.MemoryLocationSet):
                continue
            name = alloc.memorylocations[0].name
            if alloc.kind == "ExternalInput":
                if name != partition_name:
                    in_names.append(name)
            elif alloc.kind == "ExternalOutput":
                out_names.append(name)
                shape = tuple(alloc.tensor_shape)
                dtype = mybir.dt.np(alloc.dtype)
                out_avals.append(jax.core.ShapedArray(shape, dtype))
                zero_outs.append(np.zeros(shape, dtype))
        self.in_names, self.out_names = in_names, out_names
        self.out_avals, self.zero_outs = out_avals, zero_outs
        all_in_names = list(in_names) + list(out_names)
        if partition_name is not None:
            all_in_names.append(partition_name)

        def _body(*args):
            operands = list(args)
            if partition_name is not None:
                operands.append(partition_id_tensor())
            outs = _bass_exec_p.bind(
                *operands,
                out_avals=tuple(out_avals),
                in_names=tuple(all_in_names),
                out_names=tuple(out_names),
                lowering_input_output_aliases=(),
                sim_require_finite=True,
                sim_require_nnan=True,
                nc=nc,
            )
            return tuple(outs)

        devices = jax.devices()[:N_CORES]
        mesh = Mesh(np.asarray(devices), ("core",))
        n_params = len(in_names)
        in_specs = (PartitionSpec("core"),) * (n_params + len(out_names))
        out_specs = (PartitionSpec("core"),) * len(out_names)
        self.fn = jax.jit(shard_map(_body, mesh=mesh, in_specs=in_specs,
                                    out_specs=out_specs, check_rep=False),
                          keep_unused=True)

    def __call__(self, in_maps):
        jax = self.jax
        concat_in = [
            np.concatenate([np.asarray(in_maps[c][name]) for c in range(N_CORES)], axis=0)
            for name in self.in_names
        ]
        concat_zeros = [np.zeros((N_CORES * z.shape[0], *z.shape[1:]), z.dtype)
                        for z in self.zero_outs]
        outs = self.fn(*concat_in, *concat_zeros)
        jax.block_until_ready(outs)
        return [
            {name: np.asarray(outs[i]).reshape(N_CORES, *self.out_avals[i].shape)[c]
             for i, name in enumerate(self.out_names)}
            for c in range(N_CORES)
        ]


def make_core_inputs(x, ternary, global_, core):
    n, half = core // 2, core % 2
    t = ternary
    g = global_
    perm = np.concatenate([np.arange(half * Lh, (half + 1) * Lh),
                           np.arange((1 - half) * Lh, (2 - half) * Lh)])
    xT = np.ascontiguousarray(x[n].T[:, perm].astype(np.float16))
    pvv = np.zeros((128, 2), np.float32)
    pvv[:, 0] = float(half)
    pvv[:, 1] = 1.0 - float(half)
    return {
        "xT": xT,
        "tern_a": np.ascontiguousarray(t.transpose(0, 2, 1).reshape(D, H * D).astype(np.float16)),
        "tern_b": np.ascontiguousarray(t.transpose(1, 2, 0).reshape(D, H * D).astype(np.float16)),
        "glT": np.ascontiguousarray(g.transpose(1, 2, 0).reshape(D, H * G).astype(np.float16)),
        "gl2": np.ascontiguousarray(
            g.transpose(2, 0, 1).reshape(H // 2, 2, G, D)
             .transpose(1, 2, 0, 3).reshape(2 * G, (H // 2) * D).astype(np.float16)),
        "pv": pvv,
    }


def get_runner(n_iter=4):
    key = ("runner", n_iter)
    if key not in _CACHE:
        nc = build_kernel(n_iter=n_iter, num_devices=N_CORES)
        _CACHE[key] = _Runner(nc)
    return _CACHE[key]


def kernel(x, mask, ternary, global_):
    x = np.asarray(x, dtype=np.float32)
    mask = np.asarray(mask)
    ternary = np.asarray(ternary, dtype=np.float32)
    global_ = np.asarray(global_, dtype=np.float32)

    run = get_runner(4)
    in_maps = [make_core_inputs(x, ternary, global_, c) for c in range(N_CORES)]
    res = run(in_maps)
    out = np.stack([np.concatenate([res[2 * n]["y"], res[2 * n + 1]["y"]], axis=0)
                    for n in range(B)])
    out = np.where((mask != 0)[..., None], out, np.float32(0.0)).astype(np.float32)
    return out


# revision 5
# speedup vs baseline: 1.4746x; 1.0127x over previous
"""Trainium2 Bass kernel v2 for nn_AbsGlobalHeadProbEncoder (MFVI message passing).

Sequence-parallel over the query (i) axis: 8 cores = 4 batch elements x 2
token-halves.  Each core computes head scores / messages only for its own 512
query rows (all 8 heads), so per-core PE work halves vs the replicated
baseline.  Cross-core per iteration:
  - ReduceScatter (fp16) of the partial msg_j [D, L] -> each core gets the
    fully-summed messages for its own token half.
  - AllGather (fp16) of the updated Qz^T half -> full qzT for the next
    iteration's scores.
Inputs are host-permuted so each core's own tokens are local columns 0:512;
tiny DVE blends (driven by a per-core parity vector) map local <-> global
block order around the collectives.

Latency hiding: scores are built in two passes (own-j columns first, which
only need local qzT, then peer-j columns) so pass 1 overlaps the AllGather
flight; all e-transposes, msg_i matmuls and the global-node tail are deferred
until after the ReduceScatter kickoff so they overlap its flight.  e is
stored in fp8 (raw exp scores are ~e^{+-0.6}, perfectly in fp8e4 range) so
all 8 heads fit in SBUF across the two passes; softmax 1/Z (x a global M=1024
message scale, removed at F_Z assembly) is folded in-place into e.
"""
import sys
import os
import contextlib

if '/opt/trn_rl_repo' not in sys.path:
    sys.path.insert(0, '/opt/trn_rl_repo')

import numpy as np
import concourse.bacc as bacc
import concourse.mybir as mybir
import concourse.tile as tile
from concourse.masks import make_identity

F32 = mybir.dt.float32
F16 = mybir.dt.float16
F8 = mybir.dt.float8e4
U16 = mybir.dt.uint16
AF = mybir.ActivationFunctionType

B = 4
L = 1024
Lh = L // 2          # own token half
D = 128
H = 8
G = 64
IC = L // 128         # 8 j-chunks
ICO = Lh // 128       # 4 own i-chunks
N_CORES = 8
MSC = 1024.0          # global message scale M

_CACHE = {}

DBG_NO_CC = os.environ.get("DBG_NO_CC", "0") == "1"


def build_kernel(n_iter=4, num_devices=8):
    groups = [[2 * i, 2 * i + 1] for i in range(num_devices // 2)]
    nc = bacc.Bacc("TRN2", target_bir_lowering=False, debug=False,
                   num_devices=num_devices)

    xT = nc.declare_dram_parameter("xT", [D, L], F16, isOutput=False)
    tern_a = nc.declare_dram_parameter("tern_a", [D, H * D], F16, isOutput=False)
    tern_b = nc.declare_dram_parameter("tern_b", [D, H * D], F16, isOutput=False)
    glT = nc.declare_dram_parameter("glT", [D, H * G], F16, isOutput=False)
    gl2 = nc.declare_dram_parameter("gl2", [2 * G, (H // 2) * D], F16, isOutput=False)
    pv = nc.declare_dram_parameter("pv", [128, 2], F32, isOutput=False)
    y = nc.declare_dram_parameter("y", [Lh, D], F32, isOutput=True)

    with tile.TileContext(nc) as tc:
        with contextlib.ExitStack() as ctx:
            singles = ctx.enter_context(tc.tile_pool(name="singles", bufs=1))
            sb = ctx.enter_context(tc.tile_pool(name="sb", bufs=2))
            hp = ctx.enter_context(tc.tile_pool(name="hp", bufs=3))
            # PSUM budget (8 banks): fh 1x2 + aux 1x2 + tps 1x2 + msgp 2x1 = 8
            fh = ctx.enter_context(tc.tile_pool(name="fh", bufs=2, space="PSUM"))
            aux = ctx.enter_context(tc.tile_pool(name="aux", bufs=2, space="PSUM"))
            msgp = ctx.enter_context(tc.tile_pool(name="msgp", bufs=1, space="PSUM"))
            dram = ctx.enter_context(tc.tile_pool(name="dram", bufs=2, space="DRAM"))

            # ---- persistent SBUF state ----
            unaryT = singles.tile([D, L], F16)
            nc.sync.dma_start(unaryT[:], xT[:])
            ta_sb = singles.tile([D, H * D], F16)
            nc.sync.dma_start(ta_sb[:], tern_a[:])
            tb_sb = singles.tile([D, H * D], F16)
            nc.sync.dma_start(tb_sb[:], tern_b[:])
            glT_sb = singles.tile([D, H * G], F16)
            nc.sync.dma_start(glT_sb[:], glT[:])
            gl2_sb = singles.tile([2 * G, (H // 2) * D], F16)
            nc.sync.dma_start(gl2_sb[:], gl2[:])
            pv_sb = singles.tile([128, 2], F32)
            nc.sync.dma_start(pv_sb[:], pv[:])
            id16 = singles.tile([128, 128], F16)
            make_identity(nc, id16[:])
            ones128 = singles.tile([128, 128], F16)
            nc.vector.memset(ones128[:], 1.0)
            qzT = singles.tile([D, L], F16)
            # raw exp of own-j scores for all heads: [i128, (h, ic, j_own)] fp16
            e_own = singles.tile([128, H * ICO * Lh], F16)
            # et (normalized transposed probs * M): [j128, (h, jc, i_own)] fp8
            et8 = singles.tile([128, H * IC * 512], F8)
            et83 = et8.rearrange("p (h jc i) -> p h jc i", h=H, jc=IC)
            # r for all j: [j128, (jc, h, a)] fp16
            r_all = singles.tile([128, IC * H * 128], F16)
            r3 = r_all.rearrange("p (jc h a) -> p jc h a", jc=IC, h=H)
            # st (s^T) for all heads: [b128, (h, i_own)] fp16
            st_all = singles.tile([128, H * Lh], F16)
            # per-head softmax scalars
            sums_own = singles.tile([128, H * ICO], F32)
            sums_peer = singles.tile([128, H * ICO], F32)
            rrM = singles.tile([128, H * ICO], F32)

            def init_softmax():
                """qzT <- softmax_D(unary)^T for all 1024 tokens."""
                ez = sb.tile([128, L], F32, tag="ez_init")
                sums = sb.tile([128, IC], F32, tag="zsums_init")
                for c in range(IC):
                    cs = slice(c * 128, (c + 1) * 128)
                    u_ps = aux.tile([128, 128], F16, tag="aux_ps")
                    nc.tensor.transpose(u_ps[:], unaryT[:, cs], id16[:])
                    nc.scalar.activation(ez[:, cs], u_ps[:], AF.Exp,
                                         accum_out=sums[:, c:c + 1])
                rz = sb.tile([128, IC], F32, tag="zrz_init")
                nc.vector.reciprocal(rz[:], sums[:])
                qz_sc = sb.tile([128, L], F16, tag="qzsc_init")
                for c in range(IC):
                    cs = slice(c * 128, (c + 1) * 128)
                    nc.vector.tensor_scalar_mul(qz_sc[:, cs], ez[:, cs], rz[:, c:c + 1])
                for c in range(IC):
                    cs = slice(c * 128, (c + 1) * 128)
                    q_ps = aux.tile([128, 128], F16, tag="aux_ps")
                    nc.tensor.transpose(q_ps[:], qz_sc[:, cs], id16[:])
                    nc.vector.tensor_copy(qzT[:, cs], q_ps[:])

            init_softmax()

            for it in range(n_iter):
                last = (it == n_iter - 1)

                # ---------- AG-independent work (own qz half only) ----------
                # r[j, (h,a)] for own j-chunks
                def build_r(jcs):
                    for jc in jcs:
                        cs = slice(jc * 128, (jc + 1) * 128)
                        r_ps = fh.tile([128, H * 128], F32, tag="fh_ps")
                        for hh in range(2):
                            nc.tensor.matmul(r_ps[:, hh * 512:(hh + 1) * 512],
                                             qzT[:, cs],
                                             tb_sb[:, hh * 512:(hh + 1) * 512])
                        eng = nc.scalar if jc % 2 == 0 else nc.vector
                        if jc % 2 == 0:
                            nc.scalar.copy(
                                r_all[:, jc * H * 128:(jc + 1) * H * 128], r_ps[:])
                        else:
                            nc.vector.tensor_copy(
                                r_all[:, jc * H * 128:(jc + 1) * H * 128], r_ps[:])

                build_r(range(ICO))

                # F_Hg + eg for own i-chunks: eg_sb [i128, (ic, h, g)]
                eg_sb = sb.tile([128, ICO * H * G], F16, tag="eg_sb")
                eg3 = eg_sb.rearrange("p (ic h g) -> p ic h g", ic=ICO, h=H)
                eg_sums = sb.tile([128, ICO * H], F32, tag="eg_sums")
                for ic in range(ICO):
                    cs = slice(ic * 128, (ic + 1) * 128)
                    hg_ps = aux.tile([128, H * G], F32, tag="aux_ps")
                    nc.tensor.matmul(hg_ps[:], qzT[:, cs], glT_sb[:])
                    nc.scalar.activation(eg_sb[:, ic * H * G:(ic + 1) * H * G],
                                         hg_ps[:], AF.Exp)
                nc.vector.reduce_sum(
                    eg_sums.rearrange("p (ic h) -> p ic h", ic=ICO),
                    eg3, axis=mybir.AxisListType.X)

                # pass 1: st, scores-own, exp-own, s for every head
                for h in range(H):
                    hs = slice(h * 128, (h + 1) * 128)
                    st_ps = aux.tile([128, Lh], F32, tag="aux_ps")
                    nc.tensor.matmul(st_ps[:], ta_sb[:, hs], qzT[:, 0:Lh])
                    st_sb = st_all[:, h * Lh:(h + 1) * Lh]
                    nc.scalar.copy(st_sb, st_ps[:])
                    for ic in range(ICO):
                        fo_ps = fh.tile([128, Lh], F32, tag="fh_ps")
                        nc.tensor.matmul(fo_ps[:],
                                         st_all[:, h * Lh + ic * 128:h * Lh + (ic + 1) * 128],
                                         qzT[:, 0:Lh])
                        col = (h * ICO + ic) * Lh
                        nc.scalar.activation(
                            e_own[:, col:col + Lh], fo_ps[:], AF.Exp,
                            accum_out=sums_own[:, h * ICO + ic:h * ICO + ic + 1])

                # ---------- AG-dependent: peer qz half ----------
                # (for it==0 qzT is fully initialized locally)
                build_r(range(ICO, IC))

                # pass 2: scores-peer + exp, then normalize e + msg_j
                msg_ps = msgp.tile([128, L], F32, tag="msg_ps")

                ep_tiles = [None] * H

                def emit_p2_scores(h):
                    e_peer = hp.tile([128, ICO * Lh], F16, tag="e_peer")
                    for ic in range(ICO):
                        fp_ps = fh.tile([128, Lh], F32, tag="fh_ps")
                        nc.tensor.matmul(fp_ps[:],
                                         st_all[:, h * Lh + ic * 128:h * Lh + (ic + 1) * 128],
                                         qzT[:, Lh:L])
                        nc.scalar.activation(
                            e_peer[:, ic * Lh:(ic + 1) * Lh], fp_ps[:], AF.Exp,
                            accum_out=sums_peer[:, h * ICO + ic:h * ICO + ic + 1])
                    ep_tiles[h] = e_peer

                def emit_p2_norm_msgj(h):
                    hs = slice(h * 128, (h + 1) * 128)
                    e_peer = ep_tiles[h]
                    so = sums_own[:, h * ICO:(h + 1) * ICO]
                    sp = sums_peer[:, h * ICO:(h + 1) * ICO]
                    eg_h_sums = eg_sums.rearrange("p (s h) -> p s h", h=H)[:, :, h]
                    tot = hp.tile([128, ICO], F32, tag="htot")
                    nc.vector.tensor_add(tot[:], so, sp)
                    tot2 = hp.tile([128, ICO], F32, tag="htot2")
                    nc.vector.tensor_add(tot2[:], tot[:], eg_h_sums)
                    rr = hp.tile([128, ICO], F32, tag="hr")
                    nc.vector.reciprocal(rr[:], tot2[:])
                    rrM_h = rrM[:, h * ICO:(h + 1) * ICO]
                    nc.vector.tensor_scalar_mul(rrM_h, rr[:], MSC)
                    # e_norm[i, (ic, j)] = Qhs[i, j] * M  (fp16)
                    e_norm = hp.tile([128, ICO * L], F16, tag="e_norm")
                    en3 = e_norm.rearrange("p (ic j) -> p ic j", ic=ICO)
                    for ic in range(ICO):
                        nc.vector.tensor_scalar_mul(
                            en3[:, ic, 0:Lh],
                            e_own[:, (h * ICO + ic) * Lh:(h * ICO + ic + 1) * Lh],
                            rrM_h[:, ic:ic + 1])
                        nc.vector.tensor_scalar_mul(
                            en3[:, ic, Lh:L],
                            e_peer[:, ic * Lh:(ic + 1) * Lh],
                            rrM_h[:, ic:ic + 1])
                    # s (raw) for this head
                    s_ps = aux.tile([128, ICO * 128], F32, tag="aux_ps")
                    for ic in range(ICO):
                        nc.tensor.matmul(s_ps[:, ic * 128:(ic + 1) * 128],
                                         qzT[:, ic * 128:(ic + 1) * 128],
                                         ta_sb[:, hs])
                    s_raw = hp.tile([128, ICO * 128], F16, tag="s_raw")
                    nc.scalar.copy(s_raw[:], s_ps[:])
                    # msg_j: out [b, j] += sum_i s_raw[i,b] * e_norm[i,j]
                    for ic in range(ICO):
                        for half in range(2):
                            nc.tensor.matmul(
                                msg_ps[:, half * 512:(half + 1) * 512],
                                s_raw[:, ic * 128:(ic + 1) * 128],
                                en3[:, ic, half * 512:(half + 1) * 512],
                                start=(h == 0 and ic == 0),
                                stop=(h == H - 1 and ic == ICO - 1),
                                skip_group_check=True)
                    # transposes of e_norm: et[h][jc][:, i] = Qhs[i, j]*M (fp8 out)
                    for jc in range(IC):
                        t_ps = aux.tile([128, 512], F16, tag="aux_ps")
                        for ic in range(ICO):
                            nc.tensor.transpose(
                                t_ps[:, ic * 128:(ic + 1) * 128],
                                en3[:, ic, jc * 128:(jc + 1) * 128],
                                id16[:])
                        dst = et8[:, (h * IC + jc) * 512:(h * IC + jc + 1) * 512]
                        if jc % 2 == 0:
                            nc.vector.tensor_copy(dst, t_ps[:])
                        else:
                            nc.scalar.copy(dst, t_ps[:])

                emit_p2_scores(0)
                for h in range(H):
                    if h + 1 < H:
                        emit_p2_scores(h + 1)
                    emit_p2_norm_msgj(h)

                # ---------- RS kickoff ----------
                bi0 = sb.tile([128, Lh], F16, tag="bi0")
                bi1 = sb.tile([128, Lh], F16, tag="bi1")
                ta0 = sb.tile([128, Lh], F16, tag="ta0")
                ta1 = sb.tile([128, Lh], F16, tag="ta1")
                tb0 = sb.tile([128, Lh], F16, tag="tb0")
                tb1 = sb.tile([128, Lh], F16, tag="tb1")
                # block0 = own*(1-p) + peer*p ; block1 = own*p + peer*(1-p)
                nc.vector.tensor_scalar_mul(ta0[:], msg_ps[:, 0:Lh], pv_sb[:, 1:2])
                nc.vector.tensor_scalar_mul(tb0[:], msg_ps[:, Lh:L], pv_sb[:, 0:1])
                nc.vector.tensor_add(bi0[:], ta0[:], tb0[:])
                nc.vector.tensor_scalar_mul(ta1[:], msg_ps[:, 0:Lh], pv_sb[:, 0:1])
                nc.vector.tensor_scalar_mul(tb1[:], msg_ps[:, Lh:L], pv_sb[:, 1:2])
                nc.vector.tensor_add(bi1[:], ta1[:], tb1[:])
                rs_in = dram.tile([2 * 128, Lh], F16, tag="rs_in")
                rs_out = dram.tile([128, Lh], F16, tag="rs_out")
                nc.sync.dma_start(rs_in[0:128, :], bi0[:])
                nc.scalar.dma_start(rs_in[128:256, :], bi1[:])
                if DBG_NO_CC:
                    nc.sync.dma_start(rs_out[:], rs_in[0:128, :])
                else:
                    nc.gpsimd.collective_compute(
                        "ReduceScatter", mybir.AluOpType.add,
                        replica_groups=groups,
                        ins=[rs_in.opt()], outs=[rs_out.opt()])

                # ---------- deferred (overlaps RS): msg_i, eg ----------
                for h in range(H):
                    # msg_i: out [a, i_own] += sum_j r[j,a] * QhsT[j,i] * M
                    for jc in range(IC):
                        nc.tensor.matmul(
                            msg_ps[:, 0:Lh],
                            r3[:, jc, h, :],
                            et83[:, h, jc, :],
                            start=(h == 0 and jc == 0), stop=False,
                            skip_group_check=True)

                # eg tail: normalize, pair-stacked transposes, matmuls
                egn_sb = sb.tile([128, ICO * H * G], F16, tag="egn_sb")
                egn3 = egn_sb.rearrange("p (ic h g) -> p ic h g", ic=ICO, h=H)
                for ic in range(ICO):
                    for h in range(H):
                        nc.vector.tensor_scalar_mul(
                            egn3[:, ic, h, :], eg3[:, ic, h, :],
                            rrM[:, h * ICO + ic:h * ICO + ic + 1])
                for k in range(H // 2):
                    egT_ps = aux.tile([128, ICO * 128], F16, tag="aux_ps")
                    for ic in range(ICO):
                        nc.tensor.transpose(
                            egT_ps[:, ic * 128:(ic + 1) * 128],
                            egn_sb[:, (ic * H + 2 * k) * G:(ic * H + 2 * k) * G + 2 * G],
                            id16[:])
                    egT_sb = hp.tile([128, ICO * 128], F16, tag="egT_sb")
                    nc.scalar.copy(egT_sb[:], egT_ps[:])
                    nc.tensor.matmul(msg_ps[:, 0:Lh],
                                     gl2_sb[:, k * 128:(k + 1) * 128],
                                     egT_sb[:],
                                     start=False, stop=(k == H // 2 - 1),
                                     skip_group_check=True)

                # ---------- RS result -> F_Z own -> z tail ----------
                msg_red = sb.tile([128, Lh], F16, tag="msg_red")
                nc.sync.dma_start(msg_red[:], rs_out[:])
                t1 = sb.tile([128, Lh], F32, tag="t1")
                nc.vector.tensor_add(t1[:], msg_ps[:, 0:Lh], msg_red[:])
                t2 = sb.tile([128, Lh], F16, tag="t2")
                nc.vector.tensor_scalar_mul(t2[:], t1[:], 1.0 / MSC)
                fz16 = sb.tile([128, Lh], F16, tag="fz16")
                nc.vector.tensor_add(fz16[:], t2[:], unaryT[:, 0:Lh])

                if last:
                    out_sb = sb.tile([128, Lh], F32, tag="zout")
                    for ic in range(ICO):
                        cs = slice(ic * 128, (ic + 1) * 128)
                        fz_ps = aux.tile([128, 128], F16, tag="aux_ps")
                        nc.tensor.transpose(fz_ps[:], fz16[:, cs], id16[:])
                        nc.scalar.copy(out_sb[:, cs], fz_ps[:])
                        nc.sync.dma_start(y[ic * 128:(ic + 1) * 128, :],
                                          out_sb[:, cs])
                else:
                    ez = sb.tile([128, Lh], F32, tag="ez")
                    zsums = sb.tile([128, ICO], F32, tag="zsums")
                    for ic in range(ICO):
                        cs = slice(ic * 128, (ic + 1) * 128)
                        fz_ps = aux.tile([128, 128], F16, tag="aux_ps")
                        nc.tensor.transpose(fz_ps[:], fz16[:, cs], id16[:])
                        nc.scalar.activation(ez[:, cs], fz_ps[:], AF.Exp,
                                             accum_out=zsums[:, ic:ic + 1])
                    rz = sb.tile([128, ICO], F32, tag="zrz")
                    nc.vector.reciprocal(rz[:], zsums[:])
                    qz_sc = sb.tile([128, Lh], F16, tag="qzsc")
                    for ic in range(ICO):
                        cs = slice(ic * 128, (ic + 1) * 128)
                        nc.vector.tensor_scalar_mul(qz_sc[:, cs], ez[:, cs],
                                                    rz[:, ic:ic + 1])
                    qzT_ps = aux.tile([128, Lh], F16, tag="aux_ps")
                    for ic in range(ICO):
                        cs = slice(ic * 128, (ic + 1) * 128)
                        nc.tensor.transpose(qzT_ps[:, cs], qz_sc[:, cs], id16[:])
                    nc.vector.tensor_copy(qzT[:, 0:Lh], qzT_ps[:])

                    # ---------- AG of own qz half ----------
                    ag_in = dram.tile([128, Lh], F16, tag="ag_in")
                    ag_out = dram.tile([2 * 128, Lh], F16, tag="ag_out")
                    nc.sync.dma_start(ag_in[:], qzT[:, 0:Lh])
                    if DBG_NO_CC:
                        nc.sync.dma_start(ag_out[0:128, :], ag_in[:])
                        nc.sync.dma_start(ag_out[128:256, :], ag_in[:])
                    else:
                        nc.gpsimd.collective_compute(
                            "AllGather", mybir.AluOpType.bypass,
                            replica_groups=groups,
                            ins=[ag_in.opt()], outs=[ag_out.opt()])
                    g0 = sb.tile([128, Lh], F16, tag="g0")
                    g1 = sb.tile([128, Lh], F16, tag="g1")
                    nc.sync.dma_start(g0[:], ag_out[0:128, :])
                    nc.scalar.dma_start(g1[:], ag_out[128:256, :])
                    # peer half = g0*p + g1*(1-p)
                    gp = sb.tile([128, Lh], F16, tag="gp")
                    gq = sb.tile([128, Lh], F16, tag="gq")
                    nc.vector.tensor_scalar_mul(gp[:], g0[:], pv_sb[:, 0:1])
                    nc.vector.tensor_scalar_mul(gq[:], g1[:], pv_sb[:, 1:2])
                    nc.vector.tensor_add(qzT[:, Lh:L], gp[:], gq[:])

    nc.compile()
    return nc


class _Runner:
    """Keeps the jitted SPMD executable alive across kernel() calls."""

    def __init__(self, nc):
        import jax
        from jax.sharding import Mesh, PartitionSpec
        from jax.experimental.shard_map import shard_map
        from concourse.bass2jax import (_bass_exec_p, install_neuronx_cc_hook,
                                        partition_id_tensor)
        install_neuronx_cc_hook()
        self.jax = jax
        in_names, out_names, out_avals, zero_outs = [], [], [], []
        partition_name = nc.partition_id_tensor.name if nc.partition_id_tensor else None
        for alloc in nc.m.functions[0].allocations:
            if not isinstance(alloc, mybir.MemoryLocationSet):
                continue
            name = alloc.memorylocations[0].name
            if alloc.kind == "ExternalInput":
                if name != partition_name:
                    in_names.append(name)
            elif alloc.kind == "ExternalOutput":
                out_names.append(name)
                shape = tuple(alloc.tensor_shape)
                dtype = mybir.dt.np(alloc.dtype)
                out_avals.append(jax.core.ShapedArray(shape, dtype))
                zero_outs.append(np.zeros(shape, dtype))
        self.in_names, self.out_names = in_names, out_names
        self.out_avals, self.zero_outs = out_avals, zero_outs
        all_in_names = list(in_names) + list(out_names)
        if partition_name is not None:
            all_in_names.append(partition_name)

        def _body(*args):
            operands = list(args)
            if partition_name is not None:
                operands.append(partition_id_tensor())
            outs = _bass_exec_p.bind(
                *operands,
                out_avals=tuple(out_avals),
                in_names=tuple(all_in_names),
                out_names=tuple(out_names),
                lowering_input_output_aliases=(),
                sim_require_finite=True,
                sim_require_nnan=True,
                nc=nc,
            )
            return tuple(outs)

        devices = jax.devices()[:N_CORES]
        mesh = Mesh(np.asarray(devices), ("core",))
        n_params = len(in_names)
        in_specs = (PartitionSpec("core"),) * (n_params + len(out_names))
        out_specs = (PartitionSpec("core"),) * len(out_names)
        self.fn = jax.jit(shard_map(_body, mesh=mesh, in_specs=in_specs,
                                    out_specs=out_specs, check_rep=False),
                          keep_unused=True)

    def __call__(self, in_maps):
        jax = self.jax
        concat_in = [
            np.concatenate([np.asarray(in_maps[c][name]) for c in range(N_CORES)], axis=0)
            for name in self.in_names
        ]
        concat_zeros = [np.zeros((N_CORES * z.shape[0], *z.shape[1:]), z.dtype)
                        for z in self.zero_outs]
        outs = self.fn(*concat_in, *concat_zeros)
        jax.block_until_ready(outs)
        return [
            {name: np.asarray(outs[i]).reshape(N_CORES, *self.out_avals[i].shape)[c]
             for i, name in enumerate(self.out_names)}
            for c in range(N_CORES)
        ]


def make_core_inputs(x, ternary, global_, core):
    n, half = core // 2, core % 2
    t = ternary
    g = global_
    perm = np.concatenate([np.arange(half * Lh, (half + 1) * Lh),
                           np.arange((1 - half) * Lh, (2 - half) * Lh)])
    xT = np.ascontiguousarray(x[n].T[:, perm].astype(np.float16))
    pvv = np.zeros((128, 2), np.float32)
    pvv[:, 0] = float(half)
    pvv[:, 1] = 1.0 - float(half)
    return {
        "xT": xT,
        "tern_a": np.ascontiguousarray(t.transpose(0, 2, 1).reshape(D, H * D).astype(np.float16)),
        "tern_b": np.ascontiguousarray(t.transpose(1, 2, 0).reshape(D, H * D).astype(np.float16)),
        "glT": np.ascontiguousarray(g.transpose(1, 2, 0).reshape(D, H * G).astype(np.float16)),
        "gl2": np.ascontiguousarray(
            g.transpose(2, 0, 1).reshape(H // 2, 2, G, D)
             .transpose(1, 2, 0, 3).reshape(2 * G, (H // 2) * D).astype(np.float16)),
        "pv": pvv,
    }


def get_runner(n_iter=4):
    key = ("runner", n_iter)
    if key not in _CACHE:
        nc = build_kernel(n_iter=n_iter, num_devices=N_CORES)
        _CACHE[key] = _Runner(nc)
    return _CACHE[key]


def kernel(x, mask, ternary, global_):
    x = np.asarray(x, dtype=np.float32)
    mask = np.asarray(mask)
    ternary = np.asarray(ternary, dtype=np.float32)
    global_ = np.asarray(global_, dtype=np.float32)

    run = get_runner(4)
    in_maps = [make_core_inputs(x, ternary, global_, c) for c in range(N_CORES)]
    res = run(in_maps)
    out = np.stack([np.concatenate([res[2 * n]["y"], res[2 * n + 1]["y"]], axis=0)
                    for n in range(B)])
    out = np.where((mask != 0)[..., None], out, np.float32(0.0)).astype(np.float32)
    return out
